# revision 30
# baseline (speedup 1.0000x reference)
"""MultiHeadDiffAttention kernel for 8 trn2 NeuronCores.

Sharding: tensor-parallel over heads (H=8, one head per core).
Per core (head h), per batch:
  qT/kT/vT = W @ x.T   [128 feat, 2048 tok]  (ec-outer accumulation, bf16)
  v [k, dh] via 16 PE transposes of vT
  scoresT[k, q] per diff-branch via row-packed PE matmuls (c=64, concurrent)
  exp on ScalarE, both branches in one [128,1024] ACTIVATE (scores ~ N(0,1):
  no max subtraction needed)
  denominators: ee pairs pre-summed on DVE, then ones-stationary matmuls
  (half the PE streams of per-kt ones-matmuls)
  uT = v-stationary matmul streaming exp at n=512
  per-q-block tail (recip -> broadcast -> combine) is software-pipelined into
  the NEXT q-block (kt==2 slot) so the PE never drains between q-blocks and
  the HAM clock gate stays warm.
Token ownership is mixed-batch: core c owns tokens [c*256,(c+1)*256) of BOTH
batches, so each AllToAll carries only real data (no zero padding) and
phase 3 splits into two halves, the first hiding under the second AllToAll.
Post-A2A the joint-head RMS reduction uses the ones-matmul trick; norm_w and
(1-dw) are folded into Wo on the host; Wo consumes oT directly.
"""

import os
import sys

import numpy as np

if "/opt/trn_rl_repo" not in sys.path:
    sys.path.insert(0, "/opt/trn_rl_repo")

B, S, E, H = 2, 2048, 1024, 8
DH = E // H          # 128
F = DH // 2          # 64
P = 128              # partitions
NCORES = 8
OWN = 256            # tokens owned per core per batch
EC = E // P          # 8 e-chunks
KC = S // P          # 16 k-chunks per batch
QBS = 512            # q-block size
QB = S // QBS        # 4 q-blocks per batch
LAG = 3              # consume lag (k-chunks) in the attention pipeline
EPS = float(np.finfo(np.float32).eps)

LAST_RESULTS = None  # BassKernelResults of the most recent run (test.py reads this)

_NC_CACHE: dict = {}


def _build(dw: float):
    import concourse.bass as bass
    import concourse.mybir as mybir
    import concourse.tile as tile
    from concourse import bacc
    from concourse.masks import make_identity

    dt = mybir.dt
    AF = mybir.ActivationFunctionType

    nc = bacc.Bacc("TRN2", target_bir_lowering=False, debug=False, num_devices=NCORES)

    xT_d = nc.dram_tensor("xT", [B, E, S], dt.bfloat16, kind="ExternalInput")
    wqT_d = nc.dram_tensor("wqT", [E, DH], dt.bfloat16, kind="ExternalInput")
    wkT_d = nc.dram_tensor("wkT", [E, DH], dt.bfloat16, kind="ExternalInput")
    wvT_d = nc.dram_tensor("wvT", [E, DH], dt.bfloat16, kind="ExternalInput")
    woT_d = nc.dram_tensor("woT", [E, E], dt.bfloat16, kind="ExternalInput")
    out_d = nc.dram_tensor("out", [B, OWN, E], dt.bfloat16, kind="ExternalOutput")

    with tile.TileContext(nc) as tc:
        with (
            tc.tile_pool(name="consts", bufs=1) as consts,
            tc.tile_pool(name="xt", bufs=1) as xtp,
            tc.tile_pool(name="proj", bufs=1) as projp,
            tc.tile_pool(name="vv", bufs=2) as vvp,
            tc.tile_pool(name="expp", bufs=6) as expp,
            tc.tile_pool(name="esum", bufs=3) as esump,
            tc.tile_pool(name="small", bufs=2) as small,
            tc.tile_pool(name="mid", bufs=2) as mid,
            tc.tile_pool(name="osb", bufs=2) as osb,
            tc.tile_pool(name="p3", bufs=1) as p3,
            tc.tile_pool(name="dram", bufs=1, space="DRAM") as dram,
        ):
            eps_t = consts.tile([P, 1], dt.float32, tag="eps")
            nc.vector.memset(eps_t, EPS)
            ones_col = consts.tile([P, 32], dt.bfloat16, tag="ones_col")
            nc.vector.memset(ones_col, 1.0)
            ones_c1 = consts.tile([1, P], dt.bfloat16, tag="ones_c1")
            nc.vector.memset(ones_c1, 1.0)
            ident = consts.tile([P, P], dt.bfloat16, tag="ident")
            make_identity(nc, ident)
            # prime the exp table set during the initial DMA wait
            scratch = consts.tile([P, 32], dt.bfloat16, tag="scratch")
            nc.scalar.activation(scratch, ones_col, AF.Exp)

            wq_sb = consts.tile([P, EC, DH], dt.bfloat16, tag="wq")
            wk_sb = consts.tile([P, EC, DH], dt.bfloat16, tag="wk")
            wv_sb = consts.tile([P, EC, DH], dt.bfloat16, tag="wv")
            for w_sb, w_d in ((wq_sb, wqT_d), (wk_sb, wkT_d), (wv_sb, wvT_d)):
                nc.sync.dma_start(
                    out=w_sb, in_=w_d.rearrange("(c p) d -> p c d", p=P)
                )
            # both batches of xT up front, chunk-wise (compute starts on chunk 0)
            xts = []
            for b in range(B):
                xt = xtp.tile([P, EC, S], dt.bfloat16, tag=f"xt{b}", name=f"xt{b}")
                xT_v = xT_d[b].rearrange("(c p) t -> c p t", p=P)
                for ec in range(EC):
                    nc.sync.dma_start(out=xt[:, ec, :], in_=xT_v[ec])
                xts.append(xt)
            wo_sb = consts.tile([P, EC, E], dt.bfloat16, tag="wo")
            nc.sync.dma_start(out=wo_sb, in_=woT_d.rearrange("(c p) e -> p c e", p=P))

            # AllToAll buffers: block d = head-h output for a token range of
            # core d. Batch 0 is one collective of OWN tokens per block;
            # batch 1 is split into two collectives of OWN//2 tokens so the
            # phase-3 tail pipelines against the transfers.
            a2a_in = [
                dram.tile([NCORES, DH, OWN], dt.bfloat16, tag=f"a2a_in{b}",
                          name=f"a2a_in{b}")
                for b in range(B)
            ]
            a2a_out = [
                dram.tile([NCORES, DH, OWN], dt.bfloat16, tag=f"a2a_out{b}",
                          name=f"a2a_out{b}")
                for b in range(B)
            ]

            last_rrow = None

            with (
                tc.tile_pool(name="psA", bufs=2, space="PSUM") as psA,
                tc.tile_pool(name="psU", bufs=1, space="PSUM") as psU,
                tc.tile_pool(name="psS", bufs=2, space="PSUM") as psS,
            ):
                for b in range(B):
                    xt = xts[b]
                    # --- projections: ec-outer accumulation, 2 PSUM tiles ---
                    qT = projp.tile([P, S], dt.bfloat16, tag="qT", name="qT")
                    kT = projp.tile([P, S], dt.bfloat16, tag="kT", name="kT")
                    vTs = projp.tile([P, S], dt.bfloat16, tag="vTs", name="vTs")
                    # q: ec-outer so matmuls start on the first xT chunk
                    ps0 = psA.tile([P, 2, QBS], dt.float32, tag="sc", name="ps0")
                    ps1 = psA.tile([P, 2, QBS], dt.float32, tag="sc", name="ps1")
                    pss = (ps0, ps1)
                    for ec in range(EC):
                        for tb in range(4):
                            nc.tensor.matmul(
                                pss[tb // 2][:, tb % 2, :],
                                lhsT=wq_sb[:, ec, :],
                                rhs=xt[:, ec, tb * QBS:(tb + 1) * QBS],
                                start=(ec == 0),
                                stop=(ec == EC - 1),
                            )
                    for tb in range(4):
                        nc.vector.tensor_copy(
                            qT[:, tb * QBS:(tb + 1) * QBS],
                            pss[tb // 2][:, tb % 2, :],
                        )
                    # k/vT: tb-outer so each 512-token chunk completes early
                    # (first q-block's scores can start after k's chunk 0)
                    v = vvp.tile([P, KC, DH], dt.bfloat16, tag="v", name="v")
                    for w_sb, dst in ((wk_sb, kT), (wv_sb, vTs)):
                        for tb in range(4):
                            ps = psA.tile([P, 2, QBS], dt.float32, tag="sc",
                                          name="ps")
                            for ec in range(EC):
                                nc.tensor.matmul(
                                    ps[:, 0, :],
                                    lhsT=w_sb[:, ec, :],
                                    rhs=xt[:, ec, tb * QBS:(tb + 1) * QBS],
                                    start=(ec == 0),
                                    stop=(ec == EC - 1),
                                )
                            nc.vector.tensor_copy(
                                dst[:, tb * QBS:(tb + 1) * QBS], ps[:, 0, :]
                            )
                            if dst is vTs:
                                # transpose this 512-chunk to v [k-tok, dh]
                                tp = psU.tile([P, 4, P], dt.bfloat16, tag="u12",
                                              name="tp")
                                for j in range(4):
                                    kt = tb * 4 + j
                                    nc.tensor.transpose(
                                        tp[:, j, :],
                                        vTs[:, kt * P:(kt + 1) * P], ident
                                    )
                                nc.vector.tensor_copy(
                                    v[:, tb * 4:(tb + 1) * 4, :], tp
                                )

                    # --- attention ---
                    pending = None

                    def emit_tail():
                        nonlocal pending
                        pb, pqb, rrowb, u12p = pending
                        pending = None
                        rps = psA.tile([P, 2, QBS], dt.float32, tag="sc", name="rps")
                        nc.tensor.matmul(rps[:, 0, :], lhsT=ones_c1,
                                         rhs=rrowb[:, 0, :])
                        nc.tensor.matmul(rps[:, 1, :], lhsT=ones_c1,
                                         rhs=rrowb[:, 1, :])
                        rr = mid.tile([P, 2, QBS], dt.bfloat16, tag="rr", name="rr")
                        nc.vector.tensor_copy(rr, rps)
                        t1 = mid.tile([P, QBS], dt.bfloat16, tag="t1", name="t1")
                        nc.vector.tensor_mul(t1, u12p[:, 0, :], rr[:, 0, :])
                        t2 = mid.tile([P, QBS], dt.bfloat16, tag="t2", name="t2")
                        nc.vector.tensor_mul(t2, u12p[:, 1, :], rr[:, 1, :])
                        oT = osb.tile([P, QBS], dt.bfloat16, tag="oT", name="oT")
                        nc.vector.tensor_add(oT, t1, t2)
                        for half in range(2):
                            nc.sync.dma_start(
                                out=a2a_in[pb][2 * pqb + half],
                                in_=oT[:, half * OWN:(half + 1) * OWN],
                            )

                    for qb in range(QB):
                        qs = slice(qb * QBS, (qb + 1) * QBS)
                        u12 = psU.tile([P, 2, QBS], dt.float32, tag="u12",
                                       name="u12")
                        ds1 = psS.tile([32, QBS], dt.float32, tag="ds", name="ds1")
                        ds2 = psS.tile([32, QBS], dt.float32, tag="ds", name="ds2")
                        ees = []
                        ess = []
                        eqs = []

                        def consume(kt):
                            if kt % 4 == 3:
                                qr = kt // 4
                                eq = eqs[qr]
                                nc.tensor.matmul(
                                    ds1, lhsT=ones_col, rhs=eq[:, 0, :],
                                    start=(qr == 0), stop=(qr == KC // 4 - 1),
                                )
                                nc.tensor.matmul(
                                    ds2, lhsT=ones_col, rhs=eq[:, 1, :],
                                    start=(qr == 0), stop=(qr == KC // 4 - 1),
                                )
                            ee = ees[kt]
                            nc.tensor.matmul(
                                u12[:, 0, :], lhsT=v[:, kt, :], rhs=ee[:, 0, :],
                                start=(kt == 0), stop=(kt == KC - 1),
                            )
                            nc.tensor.matmul(
                                u12[:, 1, :], lhsT=v[:, kt, :], rhs=ee[:, 1, :],
                                start=(kt == 0), stop=(kt == KC - 1),
                            )

                        for kt in range(KC):
                            ks = slice(kt * P, (kt + 1) * P)
                            s12 = psA.tile([P, 2, QBS], dt.float32, tag="sc",
                                           name="s12")
                            nc.tensor.matmul(s12[:, 0, :], lhsT=kT[0:F, ks],
                                             rhs=qT[0:F, qs])
                            nc.tensor.matmul(s12[:, 1, :], lhsT=kT[F:P, ks],
                                             rhs=qT[F:P, qs])
                            ee = expp.tile([P, 2, QBS], dt.bfloat16, tag="ee",
                                           name="ee")
                            nc.scalar.activation(ee, s12, AF.Exp, scale=F**-0.5)
                            ees.append(ee)
                            if kt % 2 == 1:
                                es = esump.tile([P, 2, QBS], dt.bfloat16,
                                                tag="es", name="es")
                                nc.vector.tensor_add(es, ees[kt - 1], ee)
                                ess.append(es)
                                if kt % 4 == 3:
                                    eq = esump.tile([P, 2, QBS], dt.bfloat16,
                                                    tag="eq", name="eq", bufs=2)
                                    nc.vector.tensor_add(eq, ess[kt // 2 - 1], es)
                                    eqs.append(eq)
                            if kt == 2 and pending is not None:
                                emit_tail()
                            if kt >= LAG:
                                consume(kt - LAG)
                        for kt in range(KC - LAG, KC):
                            consume(kt)

                        rrow = small.tile([1, 2, QBS], dt.float32, tag="rrow",
                                          name="rrow")
                        nc.vector.reciprocal_approx_fast(rrow[:, 0, :],
                                                         ds1[0:1, :])
                        nc.vector.reciprocal_approx_fast(rrow[:, 1, :],
                                                         ds2[0:1, :])
                        # cast to bf16 (fold -dw into branch 2) so the
                        # broadcast matmuls are 1-pass bf16, not 2-pass fp32
                        rrowb = small.tile([1, 2, QBS], dt.bfloat16, tag="rrowb",
                                           name="rrowb")
                        nc.vector.tensor_copy(rrowb[:, 0, :], rrow[:, 0, :])
                        nc.vector.tensor_scalar_mul(rrowb[:, 1, :],
                                                    rrow[:, 1, :], -dw)
                        if b == B - 1 and qb == QB - 1:
                            last_rrow = rrow
                        pending = (b, qb, rrowb, u12)
                        if qb == QB - 1:
                            emit_tail()

                    nc.gpsimd.collective_compute(
                        "AllToAll",
                        mybir.AluOpType.bypass,
                        replica_groups=[list(range(NCORES))],
                        ins=[a2a_in[b].opt()],
                        outs=[a2a_out[b].opt()],
                    )

                # --- phase 3: RMS norm + Wo projection, four 128-token
                # quarter-pipelines. The Sqrt activations must not be hoisted
                # between attention exps (each would thrash the ACT table set,
                # ~3us); eps_live data-depends on the LAST q-block's
                # reciprocal row, pinning them after attention.
                eps_live = small.tile([1, 1], dt.float32, tag="eps_live",
                                      name="eps_live")
                nc.vector.tensor_scalar(
                    out=eps_live, in0=last_rrow[0:1, 0, 0:1],
                    scalar1=0.0, scalar2=EPS,
                    op0=mybir.AluOpType.mult, op1=mybir.AluOpType.add,
                )

                def phase3_quarter(qn, src_ap, warmers):
                    hb, qh = qn // 2, qn % 2
                    oTq = p3.tile([P, H, P], dt.bfloat16, tag="oTq", bufs=4,
                                  name="oTq")
                    nc.sync.dma_start(out=oTq, in_=src_ap)
                    sq = p3.tile([P, H, P], dt.bfloat16, tag="sq", bufs=2,
                                 name="sq")
                    nc.vector.tensor_mul(sq, oTq, oTq)
                    ssq = psS.tile([32, QBS], dt.float32, tag="ds", name="ssq")
                    for fc in range(EC):
                        nc.tensor.matmul(
                            ssq[:, :P], lhsT=ones_col, rhs=sq[:, fc, :],
                            start=(fc == 0), stop=(fc == EC - 1),
                        )
                    srow = small.tile([1, P], dt.float32, tag="srow", name="srow")
                    nc.vector.tensor_copy(srow, ssq[0:1, :P])
                    sroot = small.tile([1, P], dt.float32, tag="sroot",
                                       name="sroot")
                    nc.scalar.activation(
                        sroot, srow, AF.Sqrt, scale=1.0 / E, bias=eps_live
                    )
                    rmsr = small.tile([1, P], dt.float32, tag="rmsr", name="rmsr")
                    nc.vector.reciprocal_approx_fast(rmsr, sroot)
                    rinvb = small.tile([1, P], dt.bfloat16, tag="rinvb",
                                       name="rinvb")
                    nc.vector.tensor_copy(rinvb, rmsr)
                    rmsps = psA.tile([P, 2, QBS], dt.float32, tag="sc",
                                     name="rmsps")
                    nc.tensor.matmul(rmsps[:, 0, :P], lhsT=ones_c1, rhs=rinvb)
                    rmsb = p3.tile([P, P], dt.bfloat16, tag="rmsb", bufs=2,
                                   name="rmsb")
                    nc.vector.tensor_copy(rmsb, rmsps[:, 0, :P])

                    nrm = p3.tile([P, H, P], dt.bfloat16, tag="nrm", bufs=2,
                                  name="nrm")
                    nc.vector.tensor_mul(
                        nrm, oTq, rmsb[:, None, :].broadcast_to([P, H, P])
                    )

                    wops = psU.tile([P, 2, QBS], dt.float32, tag="u12",
                                    name="wops")
                    for fc in range(EC):
                        for nb in range(2):
                            nc.tensor.matmul(
                                wops[:, nb, :],
                                lhsT=nrm[:, fc, :],
                                rhs=wo_sb[:, fc, nb * QBS:(nb + 1) * QBS],
                                start=(fc == 0),
                                stop=(fc == EC - 1),
                            )
                    if warmers:
                        # chained junk matmuls that keep the PE's HAM clock
                        # warm while the batch-1 collectives land; gated on
                        # rmsb so they cannot be hoisted into attention
                        wps = psS.tile([32, QBS], dt.float32, tag="ds",
                                       name="wps")
                        for wi in range(warmers):
                            nc.tensor.matmul(
                                wps, lhsT=ones_col, rhs=nrm[:, 0:4, :],
                                start=(wi == 0), stop=(wi == warmers - 1),
                            )
                    out_sb = p3.tile([P, E], dt.bfloat16, tag="out_sb", bufs=2,
                                     name="out_sb")
                    nc.vector.tensor_copy(
                        out_sb.rearrange("p (n q) -> p n q", n=2), wops
                    )
                    out_v = out_d[hb].rearrange("(t p) e -> t p e", p=P)
                    nc.sync.dma_start(out=out_v[qh], in_=out_sb)

                a2a0v = a2a_out[0].rearrange("h p t -> p h t")
                a2a1v = a2a_out[1].rearrange("h p t -> p h t")
                phase3_quarter(0, a2a0v[:, :, 0:P], 0)
                phase3_quarter(1, a2a0v[:, :, P:OWN], 30)
                phase3_quarter(2, a2a1v[:, :, 0:P], 0)
                phase3_quarter(3, a2a1v[:, :, P:OWN], 0)

    nc.compile()
    return nc


def _get_nc(dw: float):
    key = round(float(dw), 9)
    if key not in _NC_CACHE:
        _NC_CACHE[key] = _build(float(dw))
    return _NC_CACHE[key]


def kernel(x, Wq, Wk, Wv, norm_w, Wo, bo, diff_weight):
    import ml_dtypes

    from concourse.bass_utils import run_bass_kernel_spmd

    global LAST_RESULTS

    bf16 = ml_dtypes.bfloat16
    x = np.asarray(x, dtype=np.float32)
    Wq = np.asarray(Wq, dtype=np.float32)
    Wk = np.asarray(Wk, dtype=np.float32)
    Wv = np.asarray(Wv, dtype=np.float32)
    Wo = np.asarray(Wo, dtype=np.float32)
    norm_w = np.asarray(norm_w, dtype=np.float32)
    bo = np.asarray(bo, dtype=np.float32)
    dw = float(np.asarray(diff_weight))

    nc = _get_nc(dw)

    xT = np.ascontiguousarray(x.transpose(0, 2, 1)).astype(bf16)  # [B, E, S]
    woT = np.ascontiguousarray(
        (Wo * norm_w.reshape(-1)[None, :] * (1.0 - dw)).T
    ).astype(bf16)  # [E(feat), E(out)]

    in_maps = []
    for h in range(NCORES):
        rows = slice(h * DH, (h + 1) * DH)
        in_maps.append(
            {
                "xT": xT,
                "wqT": np.ascontiguousarray(Wq[rows, :].T).astype(bf16),
                "wkT": np.ascontiguousarray(Wk[rows, :].T).astype(bf16),
                "wvT": np.ascontiguousarray(Wv[rows, :].T).astype(bf16),
                "woT": woT,
            }
        )

    res = run_bass_kernel_spmd(
        nc,
        in_maps,
        core_ids=list(range(NCORES)),
        trace=bool(os.environ.get("KERNEL_TRACE")),
    )
    LAST_RESULTS = res

    full = np.empty((B, S, E), dtype=np.float32)
    for c in range(NCORES):
        o = np.asarray(res.results[c]["out"], dtype=np.float32)  # [B, OWN, E]
        for b in range(B):
            full[b, c * OWN:(c + 1) * OWN, :] = o[b]
    full = full + (1.0 - dw) * bo[None, None, :]
    return full


if __name__ == "__main__":
    rng = np.random.default_rng(0)
    sc = E**-0.5
    ins = {
        "x": rng.standard_normal((B, S, E), dtype=np.float32),
        "Wq": rng.standard_normal((E, E), dtype=np.float32) * sc,
        "Wk": rng.standard_normal((E, E), dtype=np.float32) * sc,
        "Wv": rng.standard_normal((E, E), dtype=np.float32) * sc,
        "norm_w": np.ones((H, DH), dtype=np.float32),
        "Wo": rng.standard_normal((E, E), dtype=np.float32) * sc,
        "bo": np.zeros((E,), dtype=np.float32),
        "diff_weight": np.float32(0.2),
    }
    out = kernel(**ins)
    print("out", out.shape, out.dtype, float(np.abs(out).max()))


# revision 33
# speedup vs baseline: 1.1420x; 1.1420x over previous
"""MultiHeadDiffAttention kernel for 8 trn2 NeuronCores.

Sharding: tensor-parallel over heads (H=8, one head per core).
Per core (head h), per batch:
  qT/kT/vT = W @ x.T   [128 feat, 2048 tok]  (ec-outer accumulation, bf16)
  v [k, dh] via 16 PE transposes of vT
  scoresT[k, q] per diff-branch via row-packed PE matmuls (c=64, concurrent)
  exp on ScalarE, both branches in one [128,1024] ACTIVATE (scores ~ N(0,1):
  no max subtraction needed)
  denominators: ee pairs pre-summed on DVE, then ones-stationary matmuls
  (half the PE streams of per-kt ones-matmuls)
  uT = v-stationary matmul streaming exp at n=512
  per-q-block tail (recip -> broadcast -> combine) is software-pipelined into
  the NEXT q-block (kt==2 slot) so the PE never drains between q-blocks and
  the HAM clock gate stays warm.
Token ownership is mixed-batch: core c owns tokens [c*256,(c+1)*256) of BOTH
batches, so each AllToAll carries only real data (no zero padding) and
phase 3 splits into two halves, the first hiding under the second AllToAll.
Post-A2A the joint-head RMS reduction uses the ones-matmul trick; norm_w and
(1-dw) are folded into Wo on the host; Wo consumes oT directly.
"""

import os
import sys

import numpy as np

if "/opt/trn_rl_repo" not in sys.path:
    sys.path.insert(0, "/opt/trn_rl_repo")

B, S, E, H = 2, 2048, 1024, 8
DH = E // H          # 128
F = DH // 2          # 64
P = 128              # partitions
NCORES = 8
OWN = 256            # tokens owned per core per batch
EC = E // P          # 8 e-chunks
KC = S // P          # 16 k-chunks per batch
QBS = 512            # q-block size
QB = S // QBS        # 4 q-blocks per batch
LAG = 3              # consume lag (k-chunks) in the attention pipeline
EPS = float(np.finfo(np.float32).eps)

LAST_RESULTS = None  # BassKernelResults of the most recent run (test.py reads this)

_NC_CACHE: dict = {}


def _build(dw: float):
    import concourse.bass as bass
    import concourse.mybir as mybir
    import concourse.tile as tile
    from concourse import bacc
    from concourse.masks import make_identity

    dt = mybir.dt
    AF = mybir.ActivationFunctionType

    nc = bacc.Bacc("TRN2", target_bir_lowering=False, debug=False, num_devices=NCORES)

    xT_d = nc.dram_tensor("xT", [B, E, S], dt.bfloat16, kind="ExternalInput")
    wqT_d = nc.dram_tensor("wqT", [E, DH], dt.bfloat16, kind="ExternalInput")
    wkT_d = nc.dram_tensor("wkT", [E, DH], dt.bfloat16, kind="ExternalInput")
    wvT_d = nc.dram_tensor("wvT", [E, DH], dt.bfloat16, kind="ExternalInput")
    woT_d = nc.dram_tensor("woT", [E, E], dt.bfloat16, kind="ExternalInput")
    out_d = nc.dram_tensor("out", [B, OWN, E], dt.bfloat16, kind="ExternalOutput")

    with tile.TileContext(nc) as tc:
        with (
            tc.tile_pool(name="consts", bufs=1) as consts,
            tc.tile_pool(name="xt", bufs=1) as xtp,
            tc.tile_pool(name="proj", bufs=1) as projp,
            tc.tile_pool(name="vv", bufs=2) as vvp,
            tc.tile_pool(name="expp", bufs=6) as expp,
            tc.tile_pool(name="esum", bufs=3) as esump,
            tc.tile_pool(name="small", bufs=2) as small,
            tc.tile_pool(name="mid", bufs=2) as mid,
            tc.tile_pool(name="osb", bufs=2) as osb,
            tc.tile_pool(name="p3", bufs=1) as p3,
            tc.tile_pool(name="dram", bufs=1, space="DRAM") as dram,
        ):
            eps_t = consts.tile([P, 1], dt.float32, tag="eps")
            nc.vector.memset(eps_t, EPS)
            ones_col = consts.tile([P, 32], dt.bfloat16, tag="ones_col")
            nc.vector.memset(ones_col, 1.0)
            ones_c1 = consts.tile([1, P], dt.bfloat16, tag="ones_c1")
            nc.vector.memset(ones_c1, 1.0)
            ident = consts.tile([P, P], dt.bfloat16, tag="ident")
            make_identity(nc, ident)
            # prime the exp table set during the initial DMA wait
            scratch = consts.tile([P, 32], dt.bfloat16, tag="scratch")
            nc.scalar.activation(scratch, ones_col, AF.Exp)

            wq_sb = consts.tile([P, EC, DH], dt.bfloat16, tag="wq")
            wk_sb = consts.tile([P, EC, DH], dt.bfloat16, tag="wk")
            wv_sb = consts.tile([P, EC, DH], dt.bfloat16, tag="wv")
            # wq first, then the batch-0 xT chunks: the first q matmul only
            # needs those two, so don't queue wk/wv posts ahead of them
            nc.sync.dma_start(
                out=wq_sb, in_=wqT_d.rearrange("(c p) d -> p c d", p=P)
            )
            xts = []
            xt = xtp.tile([P, EC, S], dt.bfloat16, tag="xt0", name="xt0")
            xT_v = xT_d[0].rearrange("(c p) t -> c p t", p=P)
            for ec in range(EC):
                nc.sync.dma_start(out=xt[:, ec, :], in_=xT_v[ec])
            xts.append(xt)
            for w_sb, w_d in ((wk_sb, wkT_d), (wv_sb, wvT_d)):
                nc.sync.dma_start(
                    out=w_sb, in_=w_d.rearrange("(c p) d -> p c d", p=P)
                )
            xt = xtp.tile([P, EC, S], dt.bfloat16, tag="xt1", name="xt1")
            xT_v = xT_d[1].rearrange("(c p) t -> c p t", p=P)
            for ec in range(EC):
                nc.sync.dma_start(out=xt[:, ec, :], in_=xT_v[ec])
            xts.append(xt)
            wo_sb = consts.tile([P, EC, E], dt.bfloat16, tag="wo")
            nc.sync.dma_start(out=wo_sb, in_=woT_d.rearrange("(c p) e -> p c e", p=P))

            # AllToAll buffers: block d = head-h output for a token range of
            # core d. Batch 0 is one collective of OWN tokens per block;
            # batch 1 is split into two collectives of OWN//2 tokens so the
            # phase-3 tail pipelines against the transfers.
            a2a_in = [
                dram.tile([NCORES, DH, OWN], dt.bfloat16, tag=f"a2a_in{b}",
                          name=f"a2a_in{b}")
                for b in range(B)
            ]
            a2a_out = [
                dram.tile([NCORES, DH, OWN], dt.bfloat16, tag=f"a2a_out{b}",
                          name=f"a2a_out{b}")
                for b in range(B)
            ]

            last_rrow = None

            with (
                tc.tile_pool(name="psA", bufs=2, space="PSUM") as psA,
                tc.tile_pool(name="psU", bufs=1, space="PSUM") as psU,
                tc.tile_pool(name="psS", bufs=2, space="PSUM") as psS,
            ):
                for b in range(B):
                    xt = xts[b]
                    # --- projections: ec-outer accumulation, 2 PSUM tiles ---
                    qT = projp.tile([P, S], dt.bfloat16, tag="qT", name="qT")
                    kT = projp.tile([P, S], dt.bfloat16, tag="kT", name="kT")
                    vTs = projp.tile([P, S], dt.bfloat16, tag="vTs", name="vTs")
                    # q: ec-outer so matmuls start on the first xT chunk
                    ps0 = psA.tile([P, 2, QBS], dt.float32, tag="sc", name="ps0")
                    ps1 = psA.tile([P, 2, QBS], dt.float32, tag="sc", name="ps1")
                    pss = (ps0, ps1)
                    for ec in range(EC):
                        for tb in range(4):
                            nc.tensor.matmul(
                                pss[tb // 2][:, tb % 2, :],
                                lhsT=wq_sb[:, ec, :],
                                rhs=xt[:, ec, tb * QBS:(tb + 1) * QBS],
                                start=(ec == 0),
                                stop=(ec == EC - 1),
                            )
                    for tb in range(4):
                        nc.vector.tensor_copy(
                            qT[:, tb * QBS:(tb + 1) * QBS],
                            pss[tb // 2][:, tb % 2, :],
                        )
                    # k/vT: tb-outer so each 512-token chunk completes early
                    # (first q-block's scores can start after k's chunk 0)
                    v = vvp.tile([P, KC, DH], dt.bfloat16, tag="v", name="v")
                    for w_sb, dst in ((wk_sb, kT), (wv_sb, vTs)):
                        for tb in range(4):
                            ps = psA.tile([P, 2, QBS], dt.float32, tag="sc",
                                          name="ps")
                            for ec in range(EC):
                                nc.tensor.matmul(
                                    ps[:, 0, :],
                                    lhsT=w_sb[:, ec, :],
                                    rhs=xt[:, ec, tb * QBS:(tb + 1) * QBS],
                                    start=(ec == 0),
                                    stop=(ec == EC - 1),
                                )
                            nc.vector.tensor_copy(
                                dst[:, tb * QBS:(tb + 1) * QBS], ps[:, 0, :]
                            )
                            if dst is vTs:
                                # transpose this 512-chunk to v [k-tok, dh]
                                tp = psU.tile([P, 4, P], dt.bfloat16, tag="u12",
                                              name="tp")
                                for j in range(4):
                                    kt = tb * 4 + j
                                    nc.tensor.transpose(
                                        tp[:, j, :],
                                        vTs[:, kt * P:(kt + 1) * P], ident
                                    )
                                nc.vector.tensor_copy(
                                    v[:, tb * 4:(tb + 1) * 4, :], tp
                                )

                    # --- attention ---

                    def finish_recips(pd):
                        # reciprocals of the softmax denominators + bf16 cast
                        # (fold -dw into branch 2) so the broadcast matmuls
                        # are 1-pass bf16
                        ds1p, ds2p = pd["ds1"], pd["ds2"]
                        rrow = small.tile([1, 2, QBS], dt.float32, tag="rrow",
                                          name="rrow")
                        nc.vector.reciprocal_approx_fast(rrow[:, 0, :],
                                                         ds1p[0:1, :])
                        nc.vector.reciprocal_approx_fast(rrow[:, 1, :],
                                                         ds2p[0:1, :])
                        rrowb = small.tile([1, 2, QBS], dt.bfloat16, tag="rrowb",
                                           name="rrowb")
                        nc.vector.tensor_copy(rrowb[:, 0, :], rrow[:, 0, :])
                        nc.vector.tensor_scalar_mul(rrowb[:, 1, :],
                                                    rrow[:, 1, :], -dw)
                        pd["rrow"] = rrow
                        pd["rrowb"] = rrowb

                    def emit_tail(pd):
                        pb, pqb, rrowb, u12p = (pd["b"], pd["qb"], pd["rrowb"],
                                                pd["u12"])
                        rps = psA.tile([P, 2, QBS], dt.float32, tag="sc", name="rps")
                        nc.tensor.matmul(rps[:, 0, :], lhsT=ones_c1,
                                         rhs=rrowb[:, 0, :])
                        nc.tensor.matmul(rps[:, 1, :], lhsT=ones_c1,
                                         rhs=rrowb[:, 1, :])
                        rr = mid.tile([P, 2, QBS], dt.bfloat16, tag="rr", name="rr")
                        nc.vector.tensor_copy(rr, rps)
                        t1 = mid.tile([P, QBS], dt.bfloat16, tag="t1", name="t1")
                        nc.vector.tensor_mul(t1, u12p[:, 0, :], rr[:, 0, :])
                        t2 = mid.tile([P, QBS], dt.bfloat16, tag="t2", name="t2")
                        nc.vector.tensor_mul(t2, u12p[:, 1, :], rr[:, 1, :])
                        oT = osb.tile([P, QBS], dt.bfloat16, tag="oT", name="oT")
                        nc.vector.tensor_add(oT, t1, t2)
                        for half in range(2):
                            nc.sync.dma_start(
                                out=a2a_in[pb][2 * pqb + half],
                                in_=oT[:, half * OWN:(half + 1) * OWN],
                            )

                    def make_consume(u12, ds1, ds2, ees, eqs, v):
                        def consume(kt):
                            if kt % 4 == 3:
                                qr = kt // 4
                                eq = eqs[qr]
                                nc.tensor.matmul(
                                    ds1, lhsT=ones_col, rhs=eq[:, 0, :],
                                    start=(qr == 0), stop=(qr == KC // 4 - 1),
                                )
                                nc.tensor.matmul(
                                    ds2, lhsT=ones_col, rhs=eq[:, 1, :],
                                    start=(qr == 0), stop=(qr == KC // 4 - 1),
                                )
                            ee = ees[kt]
                            nc.tensor.matmul(
                                u12[:, 0, :], lhsT=v[:, kt, :], rhs=ee[:, 0, :],
                                start=(kt == 0), stop=(kt == KC - 1),
                            )
                            nc.tensor.matmul(
                                u12[:, 1, :], lhsT=v[:, kt, :], rhs=ee[:, 1, :],
                                start=(kt == 0), stop=(kt == KC - 1),
                            )
                        return consume

                    pend = None
                    for qb in range(QB):
                        qs = slice(qb * QBS, (qb + 1) * QBS)
                        u12 = psU.tile([P, 2, QBS], dt.float32, tag="u12",
                                       name="u12")
                        ds1 = psS.tile([32, QBS], dt.float32, tag="ds", name="ds1")
                        ds2 = psS.tile([32, QBS], dt.float32, tag="ds", name="ds2")
                        ees = []
                        ess = []
                        eqs = []
                        consume = make_consume(u12, ds1, ds2, ees, eqs, v)

                        for kt in range(KC):
                            ks = slice(kt * P, (kt + 1) * P)
                            s12 = psA.tile([P, 2, QBS], dt.float32, tag="sc",
                                           name="s12")
                            nc.tensor.matmul(s12[:, 0, :], lhsT=kT[0:F, ks],
                                             rhs=qT[0:F, qs])
                            nc.tensor.matmul(s12[:, 1, :], lhsT=kT[F:P, ks],
                                             rhs=qT[F:P, qs])
                            ee = expp.tile([P, 2, QBS], dt.bfloat16, tag="ee",
                                           name="ee")
                            nc.scalar.activation(ee, s12, AF.Exp, scale=F**-0.5)
                            ees.append(ee)
                            if kt % 2 == 1:
                                es = esump.tile([P, 2, QBS], dt.bfloat16,
                                                tag="es", name="es")
                                nc.vector.tensor_add(es, ees[kt - 1], ee)
                                ess.append(es)
                                if kt % 4 == 3:
                                    eq = esump.tile([P, 2, QBS], dt.bfloat16,
                                                    tag="eq", name="eq", bufs=2)
                                    nc.vector.tensor_add(eq, ess[kt // 2 - 1], es)
                                    eqs.append(eq)
                            # previous q-block's trailing consumes interleave
                            # with this block's first scores so ScalarE never
                            # drains at the boundary
                            if pend is not None:
                                if kt <= 2:
                                    pend["consume"](KC - LAG + kt)
                                if kt == 2:
                                    finish_recips(pend)
                                if kt == 3:
                                    emit_tail(pend)
                                    pend = None
                            if kt >= LAG:
                                consume(kt - LAG)
                        pend = {"consume": consume, "ds1": ds1, "ds2": ds2,
                                "u12": u12, "b": b, "qb": qb}
                        if qb == QB - 1:
                            for kt in range(KC - LAG, KC):
                                consume(kt)
                            finish_recips(pend)
                            if b == B - 1:
                                last_rrow = pend["rrow"]
                            emit_tail(pend)
                            pend = None

                    nc.gpsimd.collective_compute(
                        "AllToAll",
                        mybir.AluOpType.bypass,
                        replica_groups=[list(range(NCORES))],
                        ins=[a2a_in[b].opt()],
                        outs=[a2a_out[b].opt()],
                    )

                # --- phase 3: RMS norm + Wo projection, four 128-token
                # quarter-pipelines. The Sqrt activations must not be hoisted
                # between attention exps (each would thrash the ACT table set,
                # ~3us); eps_live data-depends on the LAST q-block's
                # reciprocal row, pinning them after attention.
                eps_live = small.tile([1, 1], dt.float32, tag="eps_live",
                                      name="eps_live")
                nc.vector.tensor_scalar(
                    out=eps_live, in0=last_rrow[0:1, 0, 0:1],
                    scalar1=0.0, scalar2=EPS,
                    op0=mybir.AluOpType.mult, op1=mybir.AluOpType.add,
                )

                def phase3_quarter(qn, src_ap, warmers):
                    hb, qh = qn // 2, qn % 2
                    oTq = p3.tile([P, H, P], dt.bfloat16, tag="oTq", bufs=4,
                                  name="oTq")
                    nc.sync.dma_start(out=oTq, in_=src_ap)
                    sq = p3.tile([P, H, P], dt.bfloat16, tag="sq", bufs=2,
                                 name="sq")
                    nc.vector.tensor_mul(sq, oTq, oTq)
                    ssq = psS.tile([32, QBS], dt.float32, tag="ds", name="ssq")
                    for fc in range(EC):
                        nc.tensor.matmul(
                            ssq[:, :P], lhsT=ones_col, rhs=sq[:, fc, :],
                            start=(fc == 0), stop=(fc == EC - 1),
                        )
                    srow = small.tile([1, P], dt.float32, tag="srow", name="srow")
                    nc.vector.tensor_copy(srow, ssq[0:1, :P])
                    sroot = small.tile([1, P], dt.float32, tag="sroot",
                                       name="sroot")
                    nc.scalar.activation(
                        sroot, srow, AF.Sqrt, scale=1.0 / E, bias=eps_live
                    )
                    rmsr = small.tile([1, P], dt.float32, tag="rmsr", name="rmsr")
                    nc.vector.reciprocal_approx_fast(rmsr, sroot)
                    rinvb = small.tile([1, P], dt.bfloat16, tag="rinvb",
                                       name="rinvb")
                    nc.vector.tensor_copy(rinvb, rmsr)
                    rmsps = psA.tile([P, 2, QBS], dt.float32, tag="sc",
                                     name="rmsps")
                    nc.tensor.matmul(rmsps[:, 0, :P], lhsT=ones_c1, rhs=rinvb)
                    rmsb = p3.tile([P, P], dt.bfloat16, tag="rmsb", bufs=2,
                                   name="rmsb")
                    nc.vector.tensor_copy(rmsb, rmsps[:, 0, :P])

                    nrm = p3.tile([P, H, P], dt.bfloat16, tag="nrm", bufs=2,
                                  name="nrm")
                    nc.vector.tensor_mul(
                        nrm, oTq, rmsb[:, None, :].broadcast_to([P, H, P])
                    )

                    wops = psU.tile([P, 2, QBS], dt.float32, tag="u12",
                                    name="wops")
                    for fc in range(EC):
                        for nb in range(2):
                            nc.tensor.matmul(
                                wops[:, nb, :],
                                lhsT=nrm[:, fc, :],
                                rhs=wo_sb[:, fc, nb * QBS:(nb + 1) * QBS],
                                start=(fc == 0),
                                stop=(fc == EC - 1),
                            )
                    if warmers:
                        # chained junk matmuls that keep the PE's HAM clock
                        # warm while the batch-1 collectives land; gated on
                        # rmsb so they cannot be hoisted into attention
                        wps = psS.tile([32, QBS], dt.float32, tag="ds",
                                       name="wps")
                        for wi in range(warmers):
                            nc.tensor.matmul(
                                wps, lhsT=ones_col, rhs=nrm[:, 0:4, :],
                                start=(wi == 0), stop=(wi == warmers - 1),
                            )
                    out_sb = p3.tile([P, E], dt.bfloat16, tag="out_sb", bufs=2,
                                     name="out_sb")
                    nc.vector.tensor_copy(
                        out_sb.rearrange("p (n q) -> p n q", n=2), wops
                    )
                    out_v = out_d[hb].rearrange("(t p) e -> t p e", p=P)
                    nc.sync.dma_start(out=out_v[qh], in_=out_sb)

                a2a0v = a2a_out[0].rearrange("h p t -> p h t")
                a2a1v = a2a_out[1].rearrange("h p t -> p h t")
                phase3_quarter(0, a2a0v[:, :, 0:P], 0)
                phase3_quarter(1, a2a0v[:, :, P:OWN], 30)
                phase3_quarter(2, a2a1v[:, :, 0:P], 0)
                phase3_quarter(3, a2a1v[:, :, P:OWN], 0)

    nc.compile()
    return nc


def _get_nc(dw: float):
    key = round(float(dw), 9)
    if key not in _NC_CACHE:
        _NC_CACHE[key] = _build(float(dw))
    return _NC_CACHE[key]


def kernel(x, Wq, Wk, Wv, norm_w, Wo, bo, diff_weight):
    import ml_dtypes

    from concourse.bass_utils import run_bass_kernel_spmd

    global LAST_RESULTS

    bf16 = ml_dtypes.bfloat16
    x = np.asarray(x, dtype=np.float32)
    Wq = np.asarray(Wq, dtype=np.float32)
    Wk = np.asarray(Wk, dtype=np.float32)
    Wv = np.asarray(Wv, dtype=np.float32)
    Wo = np.asarray(Wo, dtype=np.float32)
    norm_w = np.asarray(norm_w, dtype=np.float32)
    bo = np.asarray(bo, dtype=np.float32)
    dw = float(np.asarray(diff_weight))

    nc = _get_nc(dw)

    xT = np.ascontiguousarray(x.transpose(0, 2, 1)).astype(bf16)  # [B, E, S]
    woT = np.ascontiguousarray(
        (Wo * norm_w.reshape(-1)[None, :] * (1.0 - dw)).T
    ).astype(bf16)  # [E(feat), E(out)]

    in_maps = []
    for h in range(NCORES):
        rows = slice(h * DH, (h + 1) * DH)
        in_maps.append(
            {
                "xT": xT,
                "wqT": np.ascontiguousarray(Wq[rows, :].T).astype(bf16),
                "wkT": np.ascontiguousarray(Wk[rows, :].T).astype(bf16),
                "wvT": np.ascontiguousarray(Wv[rows, :].T).astype(bf16),
                "woT": woT,
            }
        )

    res = run_bass_kernel_spmd(
        nc,
        in_maps,
        core_ids=list(range(NCORES)),
        trace=bool(os.environ.get("KERNEL_TRACE")),
    )
    LAST_RESULTS = res

    full = np.empty((B, S, E), dtype=np.float32)
    for c in range(NCORES):
        o = np.asarray(res.results[c]["out"], dtype=np.float32)  # [B, OWN, E]
        for b in range(B):
            full[b, c * OWN:(c + 1) * OWN, :] = o[b]
    full = full + (1.0 - dw) * bo[None, None, :]
    return full


if __name__ == "__main__":
    rng = np.random.default_rng(0)
    sc = E**-0.5
    ins = {
        "x": rng.standard_normal((B, S, E), dtype=np.float32),
        "Wq": rng.standard_normal((E, E), dtype=np.float32) * sc,
        "Wk": rng.standard_normal((E, E), dtype=np.float32) * sc,
        "Wv": rng.standard_normal((E, E), dtype=np.float32) * sc,
        "norm_w": np.ones((H, DH), dtype=np.float32),
        "Wo": rng.standard_normal((E, E), dtype=np.float32) * sc,
        "bo": np.zeros((E,), dtype=np.float32),
        "diff_weight": np.float32(0.2),
    }
    out = kernel(**ins)
    print("out", out.shape, out.dtype, float(np.abs(out).max()))
